# revision 8
# baseline (speedup 1.0000x reference)
"""Trainium2 Bass kernel for nn_CAPMemory (camera-aware proxy memory loss).

Strategy (8 NeuronCores, SPMD, no collectives):
  - Shard the 64000x256 proxy/center table over P: core k owns labels
    [1000k, 1000(k+1)), columns stored CAM-MAJOR (col = cam*1000 + local
    label) and transposed on the host to [2, 128, 8000] bf16, so the
    per-camera intra softmax reads are fully contiguous and the DMA is half
    the fp32 size.
  - Feats are normalized, row-permuted (camera groups contiguous, big+small
    paired so most 128-row tiles span ~2 cameras), transposed and cast to
    bf16 on the host: no device-side transposes, norms, or sqrt-table loads.
  - Each core computes its [512, 8000] slice of the cosine matrix on the PE
    (bf16 operands, fp32 PSUM accumulate) and reduces it ON PSUM:
      cand [512, 4*8]  top-8 of each 2000-col chunk (DVE InstMax straight
                       from PSUM; no PSUM->SBUF copies anywhere)
      srow [512, 8]    sum_l exp(20*cos) for each camera present in the
                       row-tile (ACT Exp+accumulate, contiguous reads)
  - Host merge: intra logsumexp = log(sum_k srow_k); global top-50 hard
    negatives from the 8x32 candidates with the positive columns removed by
    value-matching; positives computed on host in f64.
  - Exactness certificate: every chunk's 8th-largest value must be <= the
    50th-largest merged candidate (minus a bf16 margin); rows violating it
    (~4 of 512 on random data) are recomputed exactly on host, so the result
    stays correct regardless of screening depth.

Engine budget per iteration per core (hardware-traced):
  DVE  16 x max8([128,4x500] PSUM)  ~35us   <- bottleneck (InstMax is 1x-only)
  PE   128 x matmul(bf16, 500 cols) ~27us   (LDWEIGHTS fully overlapped)
  ACT  ~10 x exp+accum(1000)        ~13us
  DMA  4.1MB centers (bf16)         ~12us

Known residual: the Tile dependency tracker serializes same-PSUM-tile
consumers even for read-read (exp waits for max8), costing ~4x1.1us of DVE
idle per iteration; splitting the max8 per bank-pair to overlap was measured
WORSE (the tracker is tile-granular, so the exp then blocks the second max8).
"""

import sys
import functools

sys.path.insert(0, "/opt/trn_rl_repo")

import numpy as np

from concourse import bacc, mybir
from concourse.tile import TileContext

F32 = mybir.dt.float32
BF16 = mybir.dt.bfloat16
BF16_NP = mybir.dt.np(mybir.dt.bfloat16)

N = 512          # batch
D = 256          # feature dim
L = 8000         # labels
C = 8            # cameras
P_LOCAL = 8000   # center columns per core (cam-major: cam*1000 + local label)
L_LOCAL = 1000   # labels per core
NCORES = 8
RT = 4           # row tiles of 128
G = 4            # column groups per core (2000 cols = 2 cameras each)
GW = 2000        # group width
CW = 500         # columns per PSUM bank (4 banks per group)
INV_T = 20.0     # 1 / temperature
K = 50           # hard negatives
LW = 0.5         # inter-cam loss weight
CAND = G * 8     # 32 candidate values per row per core
EPS_MATCH = 2.5e-3   # positive value-match tolerance (bf16 matmul error)
CERT_MARGIN = 2e-3   # certificate slack for bf16 rounding


def _pair_order(sizes):
    """Pair cameras so pair sizes are as close to 128 as possible (greedy
    best-match on the remaining set), and place the best-fitting pairs
    first: row tiles then span the fewest distinct cameras, minimizing
    intra exp instructions (each costs ~1.1us of serialized ACT time)."""
    sizes = np.asarray(sizes)
    remaining = sorted(range(C), key=lambda c: -sizes[c])
    pairs = []
    while remaining:
        a = remaining.pop(0)
        b = min(remaining, key=lambda c: abs(int(sizes[a] + sizes[c]) - 128))
        remaining.remove(b)
        pairs.append((int(a), int(b)))
    pairs.sort(key=lambda p: abs(int(sizes[p[0]] + sizes[p[1]]) - 128))
    order = []
    for a, b in pairs:
        order += [a, b]
    return order


@functools.lru_cache(maxsize=8)
def _build_program(tile_cams, repeats=1):
    nc = bacc.Bacc(None, target_bir_lowering=False, num_swdge_queues=4)

    cenT = nc.dram_tensor("cenT", [2, 128, P_LOCAL], BF16, kind="ExternalInput")
    fTd = nc.dram_tensor("fT", [2, 128, N], BF16, kind="ExternalInput")
    candd = nc.dram_tensor("cand", [RT, 128, CAND], F32, kind="ExternalOutput")
    srowd = nc.dram_tensor("srow", [RT, 128, C], F32, kind="ExternalOutput")

    with TileContext(nc) as tc:
        with (
            tc.tile_pool(name="cen", bufs=2) as cenp,
            tc.tile_pool(name="ftp", bufs=2) as ftp,
            tc.tile_pool(name="scrp", bufs=2) as scrp,
            tc.tile_pool(name="outp", bufs=2) as outp,
            tc.tile_pool(name="psum", bufs=2, space="PSUM") as psump,
        ):
            for _rep in range(repeats):
                _kernel_body(nc, tc, cenp, ftp, scrp, outp, psump,
                             cenT, fTd, candd, srowd, tile_cams)

    nc.compile()
    return nc


def _kernel_body(nc, tc, cenp, ftp, scrp, outp, psump,
                 cenT, fTd, candd, srowd, tile_cams):
    ActF = mybir.ActivationFunctionType

    # feats (tiny) first so the matmul pipeline can start immediately
    fT = ftp.tile([128, 2, N], BF16, name="fT")
    for kh in range(2):
        nc.sync.dma_start(out=fT[:, kh, :], in_=fTd[kh])
    # preload the Exp LUT before the first real activation
    warm = scrp.tile([128, 1], F32, name="warm")
    nc.scalar.activation(warm[:, 0:1], fT[:, 0, 0:1], ActF.Exp)

    # centers: per (group, half) slices, alternating issue queues so the
    # first group's columns land quickly and transfers spread across engines
    cen = cenp.tile([128, 2, P_LOCAL], BF16, name="cen")
    dma_engines = [nc.sync, nc.gpsimd]
    for g in range(G):
        s = slice(g * GW, (g + 1) * GW)
        for kh in range(2):
            dma_engines[(2 * g + kh) % 2].dma_start(
                out=cen[:, kh, s], in_=cenT[kh, :, s]
            )

    # separate tensors for candidates (DVE-written) and exp accumulators
    # (ACT-written): sharing one tile would WAW-couple max8(g,t) to the
    # previous group's exp through the dependency tracker
    cand_ts = [outp.tile([128, CAND], F32, name=f"cand{t}") for t in range(RT)]
    s_ts = [outp.tile([128, C], F32, name=f"s_t{t}") for t in range(RT)]

    # group-major: all 4 row-tiles consume group g's centers while group
    # g+1's DMA is still in flight; PSUM tiles double-buffer (4 banks each)
    for g in range(G):
        for t in range(RT):
            ps = psump.tile([128, 4, 512], F32, name="ps")
            for mk in range(4):
                cs = slice(g * GW + mk * CW, g * GW + (mk + 1) * CW)
                nc.tensor.matmul(
                    ps[:, mk, 0:CW], fT[:, 0, 128 * t : 128 * (t + 1)],
                    cen[:, 0, cs], start=True, stop=False,
                )
                nc.tensor.matmul(
                    ps[:, mk, 0:CW], fT[:, 1, 128 * t : 128 * (t + 1)],
                    cen[:, 1, cs], start=False, stop=True,
                )
            # inter screen: top-8 of the 2000 valid columns, straight off PSUM
            nc.vector.max(cand_ts[t][:, 8 * g : 8 * g + 8], ps[:, :, 0:CW])
            # intra: cams 2g (banks 0-1) and 2g+1 (banks 2-3) are contiguous
            # 1000-col blocks; one Exp+accumulate per camera present in tile
            for cam in (2 * g, 2 * g + 1):
                if cam in tile_cams[t]:
                    slot = tile_cams[t].index(cam)
                    b0 = 2 * (cam % 2)
                    scr = scrp.tile([128, 1000], BF16, name="scr")
                    nc.scalar.activation(
                        scr[:, :], ps[:, b0 : b0 + 2, 0:CW], ActF.Exp,
                        scale=INV_T,
                        accum_out=s_ts[t][:, slot : slot + 1],
                    )
            if g == G - 1:
                # outputs issue from the ACT queue: keeps the sync/gpsimd
                # queues free for the next repeat's center transfers
                nc.scalar.dma_start(out=candd[t], in_=cand_ts[t][:, :])
                nc.scalar.dma_start(out=srowd[t], in_=s_ts[t][:, :])


class _Runner:
    """Sharded 8-core executor for a built Bass program.

    Builds the jax.jit(shard_map(bass_exec)) executable once (the walrus/NEFF
    compile happens inside the first call) and reuses it for every subsequent
    execution, keeping large inputs device-resident.
    """

    def __init__(self, nc, n_cores=NCORES):
        import jax
        from jax.sharding import Mesh, PartitionSpec, NamedSharding
        from jax.experimental.shard_map import shard_map
        from concourse import bass2jax

        self.jax = jax
        self.nc = nc
        self.n_cores = n_cores
        bass2jax.install_neuronx_cc_hook()
        partition_name = (
            nc.partition_id_tensor.name if nc.partition_id_tensor else None
        )
        in_names, out_names, out_avals = [], [], []
        for alloc in nc.m.functions[0].allocations:
            if not isinstance(alloc, mybir.MemoryLocationSet):
                continue
            name = alloc.memorylocations[0].name
            if alloc.kind == "ExternalInput":
                if name != partition_name:
                    in_names.append(name)
            elif alloc.kind == "ExternalOutput":
                out_names.append(name)
                out_avals.append(
                    jax.core.ShapedArray(
                        tuple(alloc.tensor_shape), mybir.dt.np(alloc.dtype)
                    )
                )
        self.in_names, self.out_names, self.out_avals = in_names, out_names, out_avals
        n_params, n_outs = len(in_names), len(out_avals)
        all_in_names = list(in_names) + list(out_names)
        if partition_name is not None:
            all_in_names.append(partition_name)

        def _body(*args):
            operands = list(args)
            if partition_name is not None:
                operands.append(bass2jax.partition_id_tensor())
            return tuple(
                bass2jax._bass_exec_p.bind(
                    *operands,
                    out_avals=tuple(out_avals),
                    in_names=tuple(all_in_names),
                    out_names=tuple(out_names),
                    lowering_input_output_aliases=(),
                    sim_require_finite=True,
                    sim_require_nnan=True,
                    nc=nc,
                )
            )

        devices = jax.devices()[:n_cores]
        self.mesh = Mesh(np.asarray(devices), ("core",))
        self.sh = NamedSharding(self.mesh, PartitionSpec("core"))
        self.fn = jax.jit(
            shard_map(
                _body,
                mesh=self.mesh,
                in_specs=(PartitionSpec("core"),) * (n_params + n_outs),
                out_specs=(PartitionSpec("core"),) * n_outs,
                check_rep=False,
            ),
            donate_argnums=tuple(range(n_params, n_params + n_outs)),
            keep_unused=True,
        )
        self._zero_shapes = [
            ((n_cores * a.shape[0], *a.shape[1:]), a.dtype) for a in out_avals
        ]

    def put_inputs(self, in_maps):
        self.dev_in = [
            self.jax.device_put(
                np.concatenate([np.asarray(m[name]) for m in in_maps], axis=0),
                self.sh,
            )
            for name in self.in_names
        ]

    def _zeros(self):
        return [
            self.jax.device_put(np.zeros(s, d), self.sh)
            for s, d in self._zero_shapes
        ]

    def execute(self):
        outs = self.fn(*self.dev_in, *self._zeros())
        self.jax.block_until_ready(outs)
        return self.unpack(outs)

    def unpack(self, outs):
        return [
            {
                name: np.asarray(outs[i]).reshape(
                    self.n_cores, *self.out_avals[i].shape
                )[c]
                for i, name in enumerate(self.out_names)
            }
            for c in range(self.n_cores)
        ]


_RUNNERS = {}
_LAST_FALLBACKS = 0
_FORCE_FALLBACK = False  # test hook: exercise the exact host fallback path


def _get_runner(nc):
    r = _RUNNERS.get(id(nc))
    if r is None:
        r = _Runner(nc)
        _RUNNERS[id(nc)] = r
    return r


def _make_in_maps(cenT_shards, feats_p):
    fn = feats_p / np.linalg.norm(feats_p, axis=1, keepdims=True)
    fT = np.ascontiguousarray(fn.T.reshape(2, 128, N)).astype(BF16_NP)
    return [{"cenT": cenT_shards[k], "fT": fT} for k in range(NCORES)]


def _host_finish(results, feats_p, labels_p, cams_p, centers, tile_cams):
    rows = np.arange(N)
    fe = feats_p.astype(np.float64)
    fn = fe / np.linalg.norm(fe, axis=1, keepdims=True)
    cen = centers.astype(np.float64)

    # positives: 8 same-label proxies per row (host, f64, exact)
    gidx = labels_p[:, None] * C + np.arange(C)[None, :]        # [512, 8]
    pos = np.einsum("rcd,rd->rc", cen[gidx], fn)                # [512, 8]

    # ---- intra: srow slot per row from the tile's camera order ----
    slot = np.zeros(N, dtype=np.int64)
    for rt in range(RT):
        for idx, cam in enumerate(tile_cams[rt]):
            sel = slice(128 * rt, 128 * (rt + 1))
            slot[sel] = np.where(cams_p[sel] == cam, idx, slot[sel])
    rt_of = rows // 128
    p_of = rows % 128
    s_k = np.stack(
        [
            results[k]["srow"].reshape(RT, 128, C)[rt_of, p_of, slot]
            for k in range(NCORES)
        ]
    ).astype(np.float64)  # [8, 512]: per-core sum_l exp(20*cos) at own cam
    lse_intra = np.log(s_k.sum(axis=0))
    loss_intra_i = lse_intra - INV_T * pos[rows, cams_p]

    # ---- inter: merge candidates, remove positives by value ----
    CR = (
        np.stack([results[k]["cand"].reshape(N, CAND) for k in range(NCORES)])
        .transpose(1, 0, 2)
        .reshape(N, NCORES * CAND)
        .astype(np.float64)
    )
    owner = labels_p // L_LOCAL
    # cam-major: positive for cam c lives in owner-core chunk c//2; just
    # value-match all 8 positives against the owner core's 32 candidates
    for i in rows:
        base = owner[i] * CAND
        vals = CR[i, base : base + CAND]
        used = np.zeros(CAND, bool)
        for pv in pos[i]:
            d = np.abs(vals - pv)
            d[used] = np.inf
            j = np.argmin(d)
            if d[j] < EPS_MATCH:
                used[j] = True
        CR[i, base : base + CAND][used] = -np.inf

    part = np.partition(CR, NCORES * CAND - K, axis=1)[:, -K:]  # top-50
    t50 = part.min(axis=1)

    # certificate: every chunk's 8th-largest (pre-removal) must clear t50
    chunk8 = np.stack(
        [results[k]["cand"].reshape(N, G, 8)[:, :, 7] for k in range(NCORES)]
    )  # [8, 512, G]
    if _FORCE_FALLBACK:
        bad = rows
    else:
        bad = np.where(chunk8.max(axis=(0, 2)) > t50 - CERT_MARGIN)[0]
    global _LAST_FALLBACKS
    _LAST_FALLBACKS = len(bad)
    if len(bad):
        sims_bad = fn[bad] @ cen.T                              # exact rows
        for bi, i in enumerate(bad):
            row = sims_bad[bi]
            row[C * labels_p[i] : C * labels_p[i] + C] = -np.inf
            part[i] = np.sort(row)[-K:]

    z = np.concatenate([pos, part], axis=1) * INV_T             # [512, 58]
    mz = z.max(axis=1)
    lse_inter = np.log(np.exp(z - mz[:, None]).sum(axis=1)) + mz
    loss_inter_i = lse_inter - INV_T * pos.mean(axis=1)

    # ---- per-camera means, summed ----
    cnt = np.bincount(cams_p, minlength=C).astype(np.float64)
    s_intra = np.bincount(cams_p, weights=loss_intra_i, minlength=C)
    s_inter = np.bincount(cams_p, weights=loss_inter_i, minlength=C)
    safe = np.maximum(cnt, 1.0)
    li = np.sum(np.where(cnt > 0, s_intra / safe, 0.0))
    le = LW * np.sum(np.where(cnt > 0, s_inter / safe, 0.0))
    return np.array([li, le], dtype=np.float32)


def _prepare(feats, indexes, label_table, cam_table, centers):
    feats = np.asarray(feats, dtype=np.float32)
    indexes = np.asarray(indexes)
    label_table = np.asarray(label_table)
    cam_table = np.asarray(cam_table)
    centers = np.asarray(centers, dtype=np.float32)

    labels = np.asarray(label_table[indexes], dtype=np.int64)
    cams = np.asarray(cam_table[indexes], dtype=np.int64)

    # permute rows so camera groups are contiguous, ordered big+small so most
    # 128-row tiles span only ~2 cameras (fewer intra exp instructions)
    sizes = np.bincount(cams, minlength=C)
    order = _pair_order(sizes)
    perm = np.concatenate([np.where(cams == c)[0] for c in order])
    feats_p = np.ascontiguousarray(feats[perm])
    labels_p = labels[perm]
    cams_p = cams[perm]
    tile_cams = tuple(
        tuple(dict.fromkeys(cams_p[128 * rt : 128 * (rt + 1)].tolist()))
        for rt in range(RT)
    )
    # per-core centers: cam-major [cam*1000 + local label], transposed to
    # [2, 128, 8000] (contraction halves x feature dims x columns), bf16
    cen_r = centers.reshape(L, C, D)
    cenT_shards = []
    for k in range(NCORES):
        blk = cen_r[k * L_LOCAL : (k + 1) * L_LOCAL]            # [1000, C, D]
        bm = np.transpose(blk, (1, 0, 2)).reshape(P_LOCAL, D)   # cam-major
        cenT_shards.append(
            np.ascontiguousarray(bm.T).reshape(2, 128, P_LOCAL).astype(BF16_NP)
        )
    return centers, tile_cams, feats_p, labels_p, cams_p, cenT_shards


def kernel(feats, indexes, label_table, cam_table, centers):
    centers, tile_cams, feats_p, labels_p, cams_p, cenT_shards = _prepare(
        feats, indexes, label_table, cam_table, centers
    )
    nc = _build_program(tile_cams)
    runner = _get_runner(nc)
    runner.put_inputs(_make_in_maps(cenT_shards, feats_p))
    results = runner.execute()
    return _host_finish(results, feats_p, labels_p, cams_p, centers, tile_cams)
